# revision 22
# baseline (speedup 1.0000x reference)
"""Trainium2 Bass kernel for CustomRoPEAttention (B=2, S=2048, H=16, Dh=128).

Sharding: 8 cores = 2 batches x 4 head-groups (4 heads/core), tensor-parallel
over heads + data-parallel over batch. Each core computes QKV projection for
its heads, RoPE, causal softmax attention, and a partial (transposed) output
projection. Host sums the 4 partials per batch + bias.

v3 design:
  - fp16 compute everywhere; PSUM accumulation stays fp32.
  - Q/K/V persist in SBUF between projection and attention (no DRAM spill).
  - V bias dropped on-device: softmax rows sum to 1, so P@(V+bv) = P@V + bv
    and bv@Wo folds into the host-side output bias.
  - Q/K bias folded into the PSUM->SBUF eviction on the scalar engine.
  - P^T produced by xbar DMA transposes (16x128 tiles, runs on the DMA
    engines) instead of PE transposes; softmax normalize split DVE/Pool.
  - Attention processed in groups of 4 query row-blocks; PV matmuls are
    software-pipelined one group behind the score matmuls so the PE never
    waits on softmax.
  - 512-wide moving operands everywhere to amortize PE sequencer dispatch.

Self-contained: hardcodes shapes from the problem spec.
"""
import math
from contextlib import ExitStack

import numpy as np

import concourse.mybir as mybir
import concourse.tile as tile
from concourse import bacc
from concourse.bass_utils import run_bass_kernel_spmd

S = 2048            # sequence
D = 2048            # hidden
NH = 16             # total heads
DH = 128            # head dim
HG = 4              # heads per core
GQ = HG * DH        # 512: per-core q/k/v feature width
B = 2
NCORES = 8
ROPE_THETA = 10000.0
SCALE = 1.0 / math.sqrt(DH)
NEG = -1.0e9
SLAB = 512          # phase-1 sequence slab width
F32 = mybir.dt.float32
F16 = mybir.dt.float16
MULT = mybir.AluOpType.mult
ADD = mybir.AluOpType.add


def build_nc(reps=1, phases=(1, 2, 3), knobs=None):
    kn = {
        "p1x": 2, "p1s": 2, "p1m": 1, "p1ps": 3, "p1vps": 2,
        "ai": 7, "atsb": 5, "sp": 2, "ctp": 2, "spw": 1024, "pvd": 4,
        "p3ps": 2, "p3s": 4,
        # engine rotations: d=DVE, p=Pool(gpsimd), a=Act(scalar)
        "vev": "dp",        # phase-1 V eviction engines
        "mask_eng": "p",    # causal-mask add engines
        "ct_ev": "p",       # ct eviction engines
        "p3_ev": "da",      # phase-3 eviction engines
        "nrm": 1.0,         # fraction of normalize on DVE (rest on Pool)
        "tp_eng": "sa",     # dma-transpose issuing engines (s=SP sync, a=Act)
        "slab": SLAB,
    }
    if knobs:
        kn.update(knobs)
    SLB = kn["slab"]
    nc = bacc.Bacc(None, target_bir_lowering=False)
    xt = nc.dram_tensor("xt", [16, 128, S], F16, kind="ExternalInput")        # x^T tiles [kc,p,s]
    wqk = nc.dram_tensor("wqk", [16, 128, 2 * GQ], F16, kind="ExternalInput")
    wv = nc.dram_tensor("wv", [16, 128, GQ], F16, kind="ExternalInput")
    wo = nc.dram_tensor("wo", [4, 128, D], F16, kind="ExternalInput")         # Wo rows tiles
    bqkt = nc.dram_tensor("bqkt", [128, 8], F32, kind="ExternalInput")        # q/k bias per (dh, mt)
    cost = nc.dram_tensor("cost", [128, S], F16, kind="ExternalInput")        # cos^T
    sinrt = nc.dram_tensor("sinrt", [128, S], F16, kind="ExternalInput")      # sin^T with rot sign
    maskd = nc.dram_tensor("maskd", [128, 128], F32, kind="ExternalInput")    # diag causal add-mask
    outt = nc.dram_tensor("outt", [16, 128, S], F16, kind="ExternalOutput")   # partial^T tiles

    def eng(c):
        return {"d": nc.vector, "p": nc.gpsimd}[c]

    def tp_issuer(c):
        return {"s": nc.sync, "a": nc.scalar}[c]

    with tile.TileContext(nc) as tc, ExitStack() as top:
        g = top.enter_context(tc.tile_pool(name="glob", bufs=1))
        tbqkt = g.tile([128, 8], F32)
        nc.sync.dma_start(out=tbqkt, in_=bqkt[:])
        tmask = g.tile([128, 128], F32)
        nc.sync.dma_start(out=tmask, in_=maskd[:])

        pers = top.enter_context(tc.tile_pool(name="pers", bufs=1))
        qk_sb = [pers.tile([128, S], F16, name=f"qk{mt}", tag=f"qk{mt}") for mt in range(2 * HG)]
        v_sb = [pers.tile([128, GQ], F16, name=f"v{st}", tag=f"v{st}") for st in range(S // 128)]

        if 1 in phases:
            # ---------------- Phase 1: QKV^T projection + RoPE (all in SBUF) --------
            with tc.tile_pool(name="p1w", bufs=1) as p1w, \
                 tc.tile_pool(name="p1x", bufs=kn["p1x"]) as p1x, \
                 tc.tile_pool(name="p1s", bufs=kn["p1s"]) as p1s, \
                 tc.tile_pool(name="p1m", bufs=kn["p1m"]) as p1m, \
                 tc.tile_pool(name="p1ps", bufs=kn["p1ps"], space="PSUM") as p1ps, \
                 tc.tile_pool(name="p1vps", bufs=kn["p1vps"], space="PSUM") as p1vps:
                # DMA issue order is critical: the pooled DMA device serializes
                # transfers, so land exactly what the first chains need first:
                # xs0 chunk 0, then wv (V-chain weights), then xs0 chunk 1,
                # then wqk, then cos/sin.
                xs0 = p1x.tile([128, 16, SLB], F16, tag="xs")
                nc.sync.dma_start(
                    out=xs0[:, :, 0:256],
                    in_=xt[:, :, 0:256].rearrange("kc p s -> p kc s"))
                twv = p1w.tile([128, 16, GQ], F16, tag="twv")
                for idx, k2 in enumerate(range(0, 16, 2)):
                    ie = nc.scalar if idx % 2 == 0 else nc.sync
                    ie.dma_start(out=twv[:, k2:k2 + 2, :],
                                 in_=wv[k2:k2 + 2].rearrange("kc p f -> p kc f"))
                nc.sync.dma_start(
                    out=xs0[:, :, 256:SLB],
                    in_=xt[:, :, 256:SLB].rearrange("kc p s -> p kc s"))
                twqk = p1w.tile([128, 16, 2 * GQ], F16, tag="twqk")
                for idx, k2 in enumerate(range(0, 16, 2)):
                    ie = nc.scalar if idx % 2 == 0 else nc.sync
                    ie.dma_start(out=twqk[:, k2:k2 + 2, :],
                                 in_=wqk[k2:k2 + 2].rearrange("kc p f -> p kc f"))
                # cos/sin needed only after the first q/k psum chain
                tcos = p1w.tile([128, S], F16, tag="tcos")
                nc.scalar.dma_start(out=tcos, in_=cost[:])
                tsin = p1w.tile([128, S], F16, tag="tsin")
                nc.sync.dma_start(out=tsin, in_=sinrt[:])
                nv = 0
                for ns in range(S // SLB):
                    sl = slice(ns * SLB, (ns + 1) * SLB)
                    if ns == 0:
                        xs = xs0
                    else:
                        xs = p1x.tile([128, 16, SLB], F16, tag="xs")
                        nc.sync.dma_start(out=xs, in_=xt[:, :, sl].rearrange("kc p s -> p kc s"))
                    # V natural tiles first (only needs wv in SBUF)
                    for st in range(SLB // 128):
                        pv = p1vps.tile([128, GQ], F32, tag="vps")
                        s0 = st * 128
                        for kc in range(16):
                            nc.tensor.matmul(pv[:], xs[:, kc, s0:s0 + 128],
                                             twv[:, kc, :], start=(kc == 0), stop=(kc == 15))
                        e = eng(kn["vev"][nv % len(kn["vev"])])
                        nv += 1
                        e.tensor_copy(out=v_sb[ns * (SLB // 128) + st][:], in_=pv[:])
                    # Q^T and K^T head tiles (mt 0..3 = q heads, 4..7 = k heads);
                    # rotate-half swaps issued per mt-pair to cut tail latency
                    qraw = p1s.tile([128, 8, SLB], F16, tag="qraw")
                    qsw = p1s.tile([128, 8, SLB], F16, tag="qsw")
                    m1 = p1m.tile([128, 8, SLB], F16, tag="m1")
                    for mt in range(2 * HG):
                        pqk = p1ps.tile([128, SLB], F32, tag="qkps")
                        for kc in range(16):
                            nc.tensor.matmul(pqk[:], twqk[:, kc, mt * 128:(mt + 1) * 128],
                                             xs[:, kc, :], start=(kc == 0), stop=(kc == 15))
                        if ns == S // SLB - 1:
                            # keep the Act queue clear at the phase boundary so
                            # the first attention exps aren't stuck behind the
                            # last slab's evictions
                            nc.vector.tensor_scalar(out=qraw[:, mt, :], in0=pqk[:],
                                                    scalar1=tbqkt[:, mt:mt + 1],
                                                    scalar2=None, op0=ADD)
                        else:
                            nc.scalar.activation(out=qraw[:, mt, :], in_=pqk[:],
                                                 func=mybir.ActivationFunctionType.Identity,
                                                 bias=tbqkt[:, mt:mt + 1])
                        nc.vector.tensor_tensor(out=m1[:, mt, :], in0=qraw[:, mt, :],
                                                in1=tcos[:, sl], op=MULT)
                        if mt % 2 == 1:
                            pr = slice(mt - 1, mt + 1)
                            nc.sync.dma_start(out=qsw[0:64, pr, :], in_=qraw[64:128, pr, :])
                            nc.sync.dma_start(out=qsw[64:128, pr, :], in_=qraw[0:64, pr, :])
                            for mt2 in (mt - 1, mt):
                                m2 = p1m.tile([128, SLB], F16, tag="m2")
                                nc.vector.tensor_tensor(out=m2[:], in0=qsw[:, mt2, :],
                                                        in1=tsin[:, sl], op=MULT)
                                nc.vector.tensor_tensor(out=qk_sb[mt2][:, sl],
                                                        in0=m1[:, mt2, :],
                                                        in1=m2[:], op=ADD)

        # ct_sb + Wo live only in phases 2/3 — allocate them from the space the
        # phase-1 pools just released.
        pers2 = top.enter_context(tc.tile_pool(name="pers2", bufs=1))
        ct_sb = {}
        for _h in range(HG):
            for _q in range(4):
                ct_sb[(_h, _q)] = pers2.tile([128, 512], F16, name=f"ct_{_h}_{_q}",
                                             tag=f"ct_{_h}_{_q}")
        two = pers2.tile([128, 4, D], F16, tag="wo")
        nc.sync.dma_start(out=two, in_=wo.rearrange("kc p f -> p kc f"))

        if 2 in phases:
            # ---------------- Phase 2: attention per head, 4-row-block groups -------
            spw = kn["spw"]
            nct = 0
            ntp = 0
            nnr = 0

            def softmax_group(h, jj, at_sb, p2a, p2d, p2sp):
                nonlocal ntp, nnr
                for ii in range(4):
                    i = 4 * jj + ii
                    ski = (i + 1) * 128
                    nchunk = (ski + spw - 1) // spw
                    ai = p2a.tile([128, S], F16, tag="ai")
                    hs = p2d.tile([128, 2], F32, tag="hs")
                    for cc in range(nchunk):
                        off = cc * spw
                        w = min(spw, ski - off)
                        sp = p2sp.tile([128, spw], F32, tag="sp")
                        for s5 in range(0, w, 512):
                            w5 = min(512, w - s5)
                            ko = off + s5
                            nc.tensor.matmul(
                                sp[:, s5:s5 + w5],
                                qk_sb[h][:, i * 128:(i + 1) * 128],
                                qk_sb[HG + h][:, ko:ko + w5],
                                start=True, stop=True)
                        if off <= i * 128 < off + w:  # diagonal block lives here
                            dd = i * 128 - off
                            me = eng(kn["mask_eng"][i % len(kn["mask_eng"])])
                            me.tensor_tensor(out=sp[:, dd:dd + 128],
                                             in0=sp[:, dd:dd + 128],
                                             in1=tmask[:], op=ADD)
                        nc.scalar.activation(out=ai[:, off:off + w], in_=sp[:, 0:w],
                                             func=mybir.ActivationFunctionType.Exp,
                                             scale=SCALE, accum_out=hs[:, cc:cc + 1])
                    for cc in range(1, nchunk):
                        nc.vector.tensor_tensor(out=hs[:, 0:1], in0=hs[:, 0:1],
                                                in1=hs[:, cc:cc + 1], op=ADD)
                    rec = p2d.tile([128, 1], F32, tag="rec")
                    nc.vector.reciprocal(out=rec[:], in_=hs[:, 0:1])
                    # normalize in place, split DVE / Pool
                    cut = int(round(ski * kn["nrm"] / 128.0)) * 128
                    cut = max(0, min(ski, cut))
                    if cut > 0:
                        nc.vector.tensor_scalar(out=ai[:, 0:cut], in0=ai[:, 0:cut],
                                                scalar1=rec[:], scalar2=None, op0=MULT)
                    if cut < ski:
                        nc.gpsimd.tensor_scalar(out=ai[:, cut:ski], in0=ai[:, cut:ski],
                                                scalar1=rec[:], scalar2=None, op0=MULT)
                    # P^T via xbar DMA transpose straight into SBUF
                    tpe = tp_issuer(kn["tp_eng"][ntp % len(kn["tp_eng"])])
                    ntp += 1
                    tpe.dma_start_transpose(
                        out=at_sb[:, 0:i + 1, ii * 128:(ii + 1) * 128],
                        in_=ai[:, 0:ski])

            def pv_group(h, jj, at_sb, p2cp):
                nonlocal nct
                ct = p2cp.tile([128, 512], F32, tag="ct")
                last = 4 * jj + 3
                for ks in range(4 * jj + 4):
                    part = max(0, (ks - 4 * jj)) * 128
                    nc.tensor.matmul(ct[:, part:512],
                                     v_sb[ks][:, h * 128:(h + 1) * 128],
                                     at_sb[:, ks, part:512],
                                     start=(ks == 0), stop=(ks == last))
                c = kn["ct_ev"][nct % len(kn["ct_ev"])]
                nct += 1
                if c == "a":
                    nc.scalar.copy(out=ct_sb[(h, jj)][:], in_=ct[:])
                else:
                    eng(c).tensor_copy(out=ct_sb[(h, jj)][:], in_=ct[:])

            n3 = 0

            def p3_strip(ncc, p3s, p3ps):
                # output projection for query columns [512*ncc, 512*ncc+512)
                nonlocal n3
                for mt in range(16):
                    op = p3ps.tile([128, 512], F32, tag="op")
                    for kh in range(4):
                        nc.tensor.matmul(op[:], two[:, kh, mt * 128:(mt + 1) * 128],
                                         ct_sb[(kh, ncc)][:],
                                         start=(kh == 0), stop=(kh == 3))
                    ob = p3s.tile([128, 512], F16, tag="ob")
                    rot = kn["p3_ev"]
                    c = rot[n3 % len(rot)]
                    n3 += 1
                    if c == "a":
                        nc.scalar.copy(out=ob[:], in_=op[:])
                    else:
                        eng(c).tensor_copy(out=ob[:], in_=op[:])
                    nc.sync.dma_start(out=outt[mt, :, ncc * 512:(ncc + 1) * 512], in_=ob[:])

            # jj-major order: all heads finish query strip jj before jj+1, so
            # output-projection strips interleave into the attention stream and
            # keep the PE busy while softmax (Act/DVE/DMA) catches up.
            with tc.tile_pool(name="p2a", bufs=kn["ai"]) as p2a, \
                 tc.tile_pool(name="p2t", bufs=kn["atsb"]) as p2t, \
                 tc.tile_pool(name="p2d", bufs=8) as p2d, \
                 tc.tile_pool(name="p3s", bufs=kn["p3s"]) as p3s, \
                 tc.tile_pool(name="p2sp", bufs=kn["sp"], space="PSUM") as p2sp, \
                 tc.tile_pool(name="p2cp", bufs=kn["ctp"], space="PSUM") as p2cp, \
                 tc.tile_pool(name="p3ps", bufs=kn["p3ps"], space="PSUM") as p3ps:
                groups = [(jj, h) for jj in range(4) for h in range(HG)]
                pend = []   # (h, jj, at_sb) awaiting PV
                n_pv = 0
                n_p3 = 0
                for jj, h in groups:
                    at_sb = p2t.tile([128, 16, 512], F16, tag="atsb")
                    softmax_group(h, jj, at_sb, p2a, p2d, p2sp)
                    pend.append((h, jj, at_sb))
                    if len(pend) > kn["pvd"]:
                        pv_group(*pend.pop(0), p2cp)
                        n_pv += 1
                    while n_p3 < n_pv // 4:
                        p3_strip(n_p3, p3s, p3ps)
                        n_p3 += 1
                for it in pend:
                    pv_group(*it, p2cp)
                    n_pv += 1
                    while n_p3 < n_pv // 4:
                        p3_strip(n_p3, p3s, p3ps)
                        n_p3 += 1
    nc.finalize()
    return nc


_NC_CACHE = {}


def _get_nc(reps=1, knobs=None):
    key = (reps, tuple(sorted(knobs.items())) if knobs else None)
    if key not in _NC_CACHE:
        _NC_CACHE[key] = build_nc(reps, knobs=knobs)
    return _NC_CACHE[key]


def _rope_tables(position_ids_b):
    pos = position_ids_b.astype(np.float32)
    inv_freq = (1.0 / (ROPE_THETA ** (np.arange(0, DH, 2, dtype=np.float32) / np.float32(DH))))
    ang = pos[:, None] * inv_freq[None, :]          # [S, 64]
    emb = np.concatenate([ang, ang], axis=-1)       # [S, 128]
    cosT = np.ascontiguousarray(np.cos(emb).T)      # [128, S]
    sinT = np.sin(emb).T
    sin_rot = np.concatenate([-sinT[0:64], sinT[64:128]], axis=0)
    return cosT.astype(np.float16), np.ascontiguousarray(sin_rot).astype(np.float16)


def _make_in_maps(inputs):
    hidden_states = np.asarray(inputs["hidden_states"], dtype=np.float32)
    position_ids = np.asarray(inputs["position_ids"])
    Wqkv = np.asarray(inputs["Wqkv"], dtype=np.float32)
    bqkv = np.asarray(inputs["bqkv"], dtype=np.float32)
    Wo = np.asarray(inputs["Wo"], dtype=np.float32)

    mask = np.triu(np.full((128, 128), NEG, dtype=np.float32), k=1)
    tabs = [_rope_tables(np.asarray(position_ids)[b]) for b in range(B)]
    xts = [np.ascontiguousarray(hidden_states[b].T).reshape(16, 128, S).astype(np.float16)
           for b in range(B)]

    in_maps = []
    for c in range(NCORES):
        b, hg = divmod(c, HG)
        qcols = slice(hg * GQ, (hg + 1) * GQ)
        kcols = slice(D + hg * GQ, D + (hg + 1) * GQ)
        vcols = slice(2 * D + hg * GQ, 2 * D + (hg + 1) * GQ)
        wqk_c = np.ascontiguousarray(
            np.concatenate([Wqkv[:, qcols], Wqkv[:, kcols]], axis=1)
        ).reshape(16, 128, 2 * GQ).astype(np.float16)
        wv_c = np.ascontiguousarray(Wqkv[:, vcols]).reshape(16, 128, GQ).astype(np.float16)
        wo_c = np.ascontiguousarray(Wo[hg * GQ:(hg + 1) * GQ, :]).reshape(4, 128, D).astype(np.float16)
        bqk_c = np.concatenate([bqkv[qcols], bqkv[kcols]]).reshape(8, 128).T
        cosT, sin_rot = tabs[b]
        in_maps.append({
            "xt": xts[b], "wqk": wqk_c, "wv": wv_c, "wo": wo_c,
            "bqkt": np.ascontiguousarray(bqk_c).astype(np.float32),
            "cost": cosT, "sinrt": sin_rot, "maskd": mask,
        })
    return in_maps


def kernel(hidden_states, position_ids, Wqkv, bqkv, Wo, bo, _reps=1, _knobs=None):
    bo = np.asarray(bo, dtype=np.float32)
    bqkv_np = np.asarray(bqkv, dtype=np.float32)
    Wo_np = np.asarray(Wo, dtype=np.float32)
    in_maps = _make_in_maps({
        "hidden_states": hidden_states, "position_ids": position_ids,
        "Wqkv": Wqkv, "bqkv": bqkv_np, "Wo": Wo_np, "bo": bo,
    })
    nc = _get_nc(_reps, _knobs)
    res = run_bass_kernel_spmd(nc, in_maps, core_ids=list(range(NCORES)))

    # v-bias folds into the output bias: P @ (V + bv) = P @ V + bv (softmax
    # rows sum to one), so out += bv @ Wo once per batch.
    bo_eff = bo + bqkv_np[2 * D:3 * D] @ Wo_np
    out = np.empty((B, S, D), dtype=np.float32)
    for b in range(B):
        acc = res.results[b * HG]["outt"].reshape(D, S).astype(np.float32).copy()
        for hg in range(1, HG):
            acc += res.results[b * HG + hg]["outt"].reshape(D, S).astype(np.float32)
        out[b] = acc.T + bo_eff[None, :]
    return out


# revision 25
# speedup vs baseline: 1.1714x; 1.1714x over previous
"""Trainium2 Bass kernel for CustomRoPEAttention (B=2, S=2048, H=16, Dh=128).

Sharding: 8 cores = 2 batches x 4 head-groups (4 heads/core), tensor-parallel
over heads + data-parallel over batch. Each core computes QKV projection for
its heads, RoPE, causal softmax attention, and a partial (transposed) output
projection. Host sums the 4 partials per batch + bias.

v3 design:
  - fp16 compute everywhere; PSUM accumulation stays fp32.
  - Q/K/V persist in SBUF between projection and attention (no DRAM spill).
  - V bias dropped on-device: softmax rows sum to 1, so P@(V+bv) = P@V + bv
    and bv@Wo folds into the host-side output bias.
  - Q/K bias folded into the PSUM->SBUF eviction on the scalar engine.
  - P^T produced by xbar DMA transposes (16x128 tiles, runs on the DMA
    engines) instead of PE transposes; softmax normalize split DVE/Pool.
  - Attention processed in groups of 4 query row-blocks; PV matmuls are
    software-pipelined one group behind the score matmuls so the PE never
    waits on softmax.
  - 512-wide moving operands everywhere to amortize PE sequencer dispatch.

Self-contained: hardcodes shapes from the problem spec.
"""
import math
from contextlib import ExitStack

import numpy as np

import concourse.mybir as mybir
import concourse.tile as tile
from concourse import bacc
from concourse.bass_utils import run_bass_kernel_spmd

S = 2048            # sequence
D = 2048            # hidden
NH = 16             # total heads
DH = 128            # head dim
HG = 4              # heads per core
GQ = HG * DH        # 512: per-core q/k/v feature width
B = 2
NCORES = 8
ROPE_THETA = 10000.0
SCALE = 1.0 / math.sqrt(DH)
NEG = -1.0e9
SLAB = 512          # phase-1 sequence slab width
F32 = mybir.dt.float32
F16 = mybir.dt.float16
MULT = mybir.AluOpType.mult
ADD = mybir.AluOpType.add


def build_nc(reps=1, phases=(1, 2, 3), knobs=None):
    kn = {
        "p1x": 2, "p1s": 2, "p1m": 1, "p1ps": 3, "p1vps": 2,
        "ai": 7, "atsb": 5, "sp": 2, "ctp": 4, "spw": 1024, "pvd": 4,
        "p3s": 4,
        # engine rotations: d=DVE, p=Pool(gpsimd), a=Act(scalar)
        "vev": "dp",        # phase-1 V eviction engines
        "mask_eng": "p",    # causal-mask add engines
        "ct_ev": "p",       # ct eviction engines
        "p3_ev": "da",      # phase-3 eviction engines
        "nrm": 1.0,         # fraction of normalize on DVE (rest on Pool)
        "tp_eng": "sa",     # dma-transpose issuing engines (s=SP sync, a=Act)
        "slab": SLAB,
    }
    if knobs:
        kn.update(knobs)
    SLB = kn["slab"]
    nc = bacc.Bacc(None, target_bir_lowering=False)
    xt = nc.dram_tensor("xt", [16, 128, S], F16, kind="ExternalInput")        # x^T tiles [kc,p,s]
    wqk = nc.dram_tensor("wqk", [16, 128, 2 * GQ], F16, kind="ExternalInput")
    wv = nc.dram_tensor("wv", [16, 128, GQ], F16, kind="ExternalInput")
    wo = nc.dram_tensor("wo", [4, 128, D], F16, kind="ExternalInput")         # Wo rows tiles
    bqkt = nc.dram_tensor("bqkt", [128, 8], F32, kind="ExternalInput")        # q/k bias per (dh, mt)
    cost = nc.dram_tensor("cost", [128, S], F16, kind="ExternalInput")        # cos^T
    sinrt = nc.dram_tensor("sinrt", [128, S], F16, kind="ExternalInput")      # sin^T with rot sign
    maskd = nc.dram_tensor("maskd", [128, 128], F32, kind="ExternalInput")    # diag causal add-mask
    outt = nc.dram_tensor("outt", [16, 128, S], F16, kind="ExternalOutput")   # partial^T tiles

    def eng(c):
        return {"d": nc.vector, "p": nc.gpsimd}[c]

    def tp_issuer(c):
        return {"s": nc.sync, "a": nc.scalar}[c]

    with tile.TileContext(nc) as tc, ExitStack() as top:
        g = top.enter_context(tc.tile_pool(name="glob", bufs=1))
        tbqkt = g.tile([128, 8], F32)
        nc.sync.dma_start(out=tbqkt, in_=bqkt[:])
        tmask = g.tile([128, 128], F32)
        nc.sync.dma_start(out=tmask, in_=maskd[:])

        pers = top.enter_context(tc.tile_pool(name="pers", bufs=1))
        qk_sb = [pers.tile([128, S], F16, name=f"qk{mt}", tag=f"qk{mt}") for mt in range(2 * HG)]
        v_sb = [pers.tile([128, GQ], F16, name=f"v{st}", tag=f"v{st}") for st in range(S // 128)]

        if 1 in phases:
            # ---------------- Phase 1: QKV^T projection + RoPE (all in SBUF) --------
            with tc.tile_pool(name="p1w", bufs=1) as p1w, \
                 tc.tile_pool(name="p1x", bufs=kn["p1x"]) as p1x, \
                 tc.tile_pool(name="p1s", bufs=kn["p1s"]) as p1s, \
                 tc.tile_pool(name="p1m", bufs=kn["p1m"]) as p1m, \
                 tc.tile_pool(name="p1ps", bufs=kn["p1ps"], space="PSUM") as p1ps, \
                 tc.tile_pool(name="p1vps", bufs=kn["p1vps"], space="PSUM") as p1vps:
                # DMA issue order is critical: the pooled DMA device serializes
                # transfers, so land exactly what the first chains need first:
                # xs0 chunk 0, then wv (V-chain weights), then xs0 chunk 1,
                # then wqk, then cos/sin.
                xs0 = p1x.tile([128, 16, SLB], F16, tag="xs")
                nc.sync.dma_start(
                    out=xs0[:, :, 0:256],
                    in_=xt[:, :, 0:256].rearrange("kc p s -> p kc s"))
                twv = p1w.tile([128, 16, GQ], F16, tag="twv")
                for idx, k2 in enumerate(range(0, 16, 2)):
                    ie = nc.scalar if idx % 2 == 0 else nc.sync
                    ie.dma_start(out=twv[:, k2:k2 + 2, :],
                                 in_=wv[k2:k2 + 2].rearrange("kc p f -> p kc f"))
                nc.sync.dma_start(
                    out=xs0[:, :, 256:SLB],
                    in_=xt[:, :, 256:SLB].rearrange("kc p s -> p kc s"))
                twqk = p1w.tile([128, 16, 2 * GQ], F16, tag="twqk")
                for idx, k2 in enumerate(range(0, 16, 2)):
                    ie = nc.scalar if idx % 2 == 0 else nc.sync
                    ie.dma_start(out=twqk[:, k2:k2 + 2, :],
                                 in_=wqk[k2:k2 + 2].rearrange("kc p f -> p kc f"))
                # cos/sin needed only after the first q/k psum chain
                tcos = p1w.tile([128, S], F16, tag="tcos")
                nc.scalar.dma_start(out=tcos, in_=cost[:])
                tsin = p1w.tile([128, S], F16, tag="tsin")
                nc.sync.dma_start(out=tsin, in_=sinrt[:])
                nv = 0
                for ns in range(S // SLB):
                    sl = slice(ns * SLB, (ns + 1) * SLB)
                    if ns == 0:
                        xs = xs0
                    else:
                        xs = p1x.tile([128, 16, SLB], F16, tag="xs")
                        nc.sync.dma_start(out=xs, in_=xt[:, :, sl].rearrange("kc p s -> p kc s"))
                    # V natural tiles first (only needs wv in SBUF)
                    for st in range(SLB // 128):
                        pv = p1vps.tile([128, GQ], F32, tag="vps")
                        s0 = st * 128
                        for kc in range(16):
                            nc.tensor.matmul(pv[:], xs[:, kc, s0:s0 + 128],
                                             twv[:, kc, :], start=(kc == 0), stop=(kc == 15))
                        e = eng(kn["vev"][nv % len(kn["vev"])])
                        nv += 1
                        e.tensor_copy(out=v_sb[ns * (SLB // 128) + st][:], in_=pv[:])
                    # Q^T and K^T head tiles (mt 0..3 = q heads, 4..7 = k heads);
                    # rotate-half swaps issued per mt-pair to cut tail latency
                    qraw = p1s.tile([128, 8, SLB], F16, tag="qraw")
                    qsw = p1s.tile([128, 8, SLB], F16, tag="qsw")
                    m1 = p1m.tile([128, 8, SLB], F16, tag="m1")
                    for mt in range(2 * HG):
                        pqk = p1ps.tile([128, SLB], F32, tag="qkps")
                        for kc in range(16):
                            nc.tensor.matmul(pqk[:], twqk[:, kc, mt * 128:(mt + 1) * 128],
                                             xs[:, kc, :], start=(kc == 0), stop=(kc == 15))
                        if ns == S // SLB - 1:
                            # keep the Act queue clear at the phase boundary so
                            # the first attention exps aren't stuck behind the
                            # last slab's evictions
                            nc.vector.tensor_scalar(out=qraw[:, mt, :], in0=pqk[:],
                                                    scalar1=tbqkt[:, mt:mt + 1],
                                                    scalar2=None, op0=ADD)
                        else:
                            nc.scalar.activation(out=qraw[:, mt, :], in_=pqk[:],
                                                 func=mybir.ActivationFunctionType.Identity,
                                                 bias=tbqkt[:, mt:mt + 1])
                        nc.vector.tensor_tensor(out=m1[:, mt, :], in0=qraw[:, mt, :],
                                                in1=tcos[:, sl], op=MULT)
                        if mt % 2 == 1:
                            pr = slice(mt - 1, mt + 1)
                            nc.sync.dma_start(out=qsw[0:64, pr, :], in_=qraw[64:128, pr, :])
                            nc.sync.dma_start(out=qsw[64:128, pr, :], in_=qraw[0:64, pr, :])
                            for mt2 in (mt - 1, mt):
                                m2 = p1m.tile([128, SLB], F16, tag="m2")
                                nc.vector.tensor_tensor(out=m2[:], in0=qsw[:, mt2, :],
                                                        in1=tsin[:, sl], op=MULT)
                                nc.vector.tensor_tensor(out=qk_sb[mt2][:, sl],
                                                        in0=m1[:, mt2, :],
                                                        in1=m2[:], op=ADD)

        # ct_sb + Wo live only in phases 2/3 — allocate them from the space the
        # phase-1 pools just released.
        pers2 = top.enter_context(tc.tile_pool(name="pers2", bufs=1))
        ct_sb = {}
        for _h in range(HG):
            for _q in range(4):
                ct_sb[(_h, _q)] = pers2.tile([128, 512], F16, name=f"ct_{_h}_{_q}",
                                             tag=f"ct_{_h}_{_q}")
        two = pers2.tile([128, 4, D], F16, tag="wo")
        nc.sync.dma_start(out=two, in_=wo.rearrange("kc p f -> p kc f"))

        if 2 in phases:
            # ---------------- Phase 2: attention per head, 4-row-block groups -------
            spw = kn["spw"]
            nct = 0
            ntp = 0
            nnr = 0

            def softmax_group(h, jj, at_sb, p2a, p2d, p2sp):
                nonlocal ntp, nnr
                for ii in range(4):
                    i = 4 * jj + ii
                    ski = (i + 1) * 128
                    nchunk = (ski + spw - 1) // spw
                    ai = p2a.tile([128, S], F16, tag="ai")
                    hs = p2d.tile([128, 2], F32, tag="hs")
                    for cc in range(nchunk):
                        off = cc * spw
                        w = min(spw, ski - off)
                        sp = p2sp.tile([128, spw], F32, tag="sp")
                        for s5 in range(0, w, 512):
                            w5 = min(512, w - s5)
                            ko = off + s5
                            nc.tensor.matmul(
                                sp[:, s5:s5 + w5],
                                qk_sb[h][:, i * 128:(i + 1) * 128],
                                qk_sb[HG + h][:, ko:ko + w5],
                                start=True, stop=True)
                        if off <= i * 128 < off + w:  # diagonal block lives here
                            dd = i * 128 - off
                            me = eng(kn["mask_eng"][i % len(kn["mask_eng"])])
                            me.tensor_tensor(out=sp[:, dd:dd + 128],
                                             in0=sp[:, dd:dd + 128],
                                             in1=tmask[:], op=ADD)
                        nc.scalar.activation(out=ai[:, off:off + w], in_=sp[:, 0:w],
                                             func=mybir.ActivationFunctionType.Exp,
                                             scale=SCALE, accum_out=hs[:, cc:cc + 1])
                    for cc in range(1, nchunk):
                        nc.vector.tensor_tensor(out=hs[:, 0:1], in0=hs[:, 0:1],
                                                in1=hs[:, cc:cc + 1], op=ADD)
                    rec = p2d.tile([128, 1], F32, tag="rec")
                    nc.vector.reciprocal(out=rec[:], in_=hs[:, 0:1])
                    # normalize in place, split DVE / Pool
                    cut = int(round(ski * kn["nrm"] / 128.0)) * 128
                    cut = max(0, min(ski, cut))
                    if cut > 0:
                        nc.vector.tensor_scalar(out=ai[:, 0:cut], in0=ai[:, 0:cut],
                                                scalar1=rec[:], scalar2=None, op0=MULT)
                    if cut < ski:
                        nc.gpsimd.tensor_scalar(out=ai[:, cut:ski], in0=ai[:, cut:ski],
                                                scalar1=rec[:], scalar2=None, op0=MULT)
                    # P^T via xbar DMA transpose straight into SBUF
                    tpe = tp_issuer(kn["tp_eng"][ntp % len(kn["tp_eng"])])
                    ntp += 1
                    tpe.dma_start_transpose(
                        out=at_sb[:, 0:i + 1, ii * 128:(ii + 1) * 128],
                        in_=ai[:, 0:ski])

            def pv_group(h, jj, at_sb, p2cp):
                nonlocal nct
                ct = p2cp.tile([128, 512], F32, tag="ctop")
                last = 4 * jj + 3
                for ks in range(4 * jj + 4):
                    part = max(0, (ks - 4 * jj)) * 128
                    nc.tensor.matmul(ct[:, part:512],
                                     v_sb[ks][:, h * 128:(h + 1) * 128],
                                     at_sb[:, ks, part:512],
                                     start=(ks == 0), stop=(ks == last))
                c = kn["ct_ev"][nct % len(kn["ct_ev"])]
                nct += 1
                if c == "a":
                    nc.scalar.copy(out=ct_sb[(h, jj)][:], in_=ct[:])
                else:
                    eng(c).tensor_copy(out=ct_sb[(h, jj)][:], in_=ct[:])

            n3 = 0

            def p3_strip(ncc, p3s, p3ps):
                # output projection for query columns [512*ncc, 512*ncc+512)
                nonlocal n3
                for mt in range(16):
                    op = p3ps.tile([128, 512], F32, tag="ctop")
                    for kh in range(4):
                        nc.tensor.matmul(op[:], two[:, kh, mt * 128:(mt + 1) * 128],
                                         ct_sb[(kh, ncc)][:],
                                         start=(kh == 0), stop=(kh == 3))
                    ob = p3s.tile([128, 512], F16, tag="ob")
                    rot = kn["p3_ev"]
                    c = rot[n3 % len(rot)]
                    n3 += 1
                    if c == "a":
                        nc.scalar.copy(out=ob[:], in_=op[:])
                    else:
                        eng(c).tensor_copy(out=ob[:], in_=op[:])
                    nc.sync.dma_start(out=outt[mt, :, ncc * 512:(ncc + 1) * 512], in_=ob[:])

            # h-major groups; output-projection strip ncc is emitted as soon
            # as its last head (h=3, jj=ncc) has been PV'd, filling the last
            # head's softmax stalls and shortening the tail.
            with tc.tile_pool(name="p2a", bufs=kn["ai"]) as p2a, \
                 tc.tile_pool(name="p2t", bufs=kn["atsb"]) as p2t, \
                 tc.tile_pool(name="p2d", bufs=8) as p2d, \
                 tc.tile_pool(name="p3s", bufs=kn["p3s"]) as p3s, \
                 tc.tile_pool(name="p2sp", bufs=kn["sp"], space="PSUM") as p2sp, \
                 tc.tile_pool(name="p2cp", bufs=kn["ctp"], space="PSUM") as p2cp:
                pend = []   # (h, jj, at_sb) awaiting PV
                for h in range(HG):
                    for jj in range(4):
                        at_sb = p2t.tile([128, 16, 512], F16, tag="atsb")
                        softmax_group(h, jj, at_sb, p2a, p2d, p2sp)
                        pend.append((h, jj, at_sb))
                        if len(pend) > kn["pvd"]:
                            ph, pjj, pat = pend.pop(0)
                            pv_group(ph, pjj, pat, p2cp)
                            if ph == 3:
                                p3_strip(pjj, p3s, p2cp)
                for ph, pjj, pat in pend:
                    pv_group(ph, pjj, pat, p2cp)
                    if ph == 3:
                        p3_strip(pjj, p3s, p2cp)
    nc.finalize()
    return nc


_NC_CACHE = {}


def _get_nc(reps=1, knobs=None):
    key = (reps, tuple(sorted(knobs.items())) if knobs else None)
    if key not in _NC_CACHE:
        _NC_CACHE[key] = build_nc(reps, knobs=knobs)
    return _NC_CACHE[key]


def _rope_tables(position_ids_b):
    pos = position_ids_b.astype(np.float32)
    inv_freq = (1.0 / (ROPE_THETA ** (np.arange(0, DH, 2, dtype=np.float32) / np.float32(DH))))
    ang = pos[:, None] * inv_freq[None, :]          # [S, 64]
    emb = np.concatenate([ang, ang], axis=-1)       # [S, 128]
    cosT = np.ascontiguousarray(np.cos(emb).T)      # [128, S]
    sinT = np.sin(emb).T
    sin_rot = np.concatenate([-sinT[0:64], sinT[64:128]], axis=0)
    return cosT.astype(np.float16), np.ascontiguousarray(sin_rot).astype(np.float16)


def _make_in_maps(inputs):
    hidden_states = np.asarray(inputs["hidden_states"], dtype=np.float32)
    position_ids = np.asarray(inputs["position_ids"])
    Wqkv = np.asarray(inputs["Wqkv"], dtype=np.float32)
    bqkv = np.asarray(inputs["bqkv"], dtype=np.float32)
    Wo = np.asarray(inputs["Wo"], dtype=np.float32)

    mask = np.triu(np.full((128, 128), NEG, dtype=np.float32), k=1)
    tabs = [_rope_tables(np.asarray(position_ids)[b]) for b in range(B)]
    xts = [np.ascontiguousarray(hidden_states[b].T).reshape(16, 128, S).astype(np.float16)
           for b in range(B)]

    in_maps = []
    for c in range(NCORES):
        b, hg = divmod(c, HG)
        qcols = slice(hg * GQ, (hg + 1) * GQ)
        kcols = slice(D + hg * GQ, D + (hg + 1) * GQ)
        vcols = slice(2 * D + hg * GQ, 2 * D + (hg + 1) * GQ)
        wqk_c = np.ascontiguousarray(
            np.concatenate([Wqkv[:, qcols], Wqkv[:, kcols]], axis=1)
        ).reshape(16, 128, 2 * GQ).astype(np.float16)
        wv_c = np.ascontiguousarray(Wqkv[:, vcols]).reshape(16, 128, GQ).astype(np.float16)
        wo_c = np.ascontiguousarray(Wo[hg * GQ:(hg + 1) * GQ, :]).reshape(4, 128, D).astype(np.float16)
        bqk_c = np.concatenate([bqkv[qcols], bqkv[kcols]]).reshape(8, 128).T
        cosT, sin_rot = tabs[b]
        in_maps.append({
            "xt": xts[b], "wqk": wqk_c, "wv": wv_c, "wo": wo_c,
            "bqkt": np.ascontiguousarray(bqk_c).astype(np.float32),
            "cost": cosT, "sinrt": sin_rot, "maskd": mask,
        })
    return in_maps


def kernel(hidden_states, position_ids, Wqkv, bqkv, Wo, bo, _reps=1, _knobs=None):
    bo = np.asarray(bo, dtype=np.float32)
    bqkv_np = np.asarray(bqkv, dtype=np.float32)
    Wo_np = np.asarray(Wo, dtype=np.float32)
    in_maps = _make_in_maps({
        "hidden_states": hidden_states, "position_ids": position_ids,
        "Wqkv": Wqkv, "bqkv": bqkv_np, "Wo": Wo_np, "bo": bo,
    })
    nc = _get_nc(_reps, _knobs)
    res = run_bass_kernel_spmd(nc, in_maps, core_ids=list(range(NCORES)))

    # v-bias folds into the output bias: P @ (V + bv) = P @ V + bv (softmax
    # rows sum to one), so out += bv @ Wo once per batch.
    bo_eff = bo + bqkv_np[2 * D:3 * D] @ Wo_np
    out = np.empty((B, S, D), dtype=np.float32)
    for b in range(B):
        acc = res.results[b * HG]["outt"].reshape(D, S).astype(np.float32).copy()
        for hg in range(1, HG):
            acc += res.results[b * HG + hg]["outt"].reshape(D, S).astype(np.float32)
        out[b] = acc.T + bo_eff[None, :]
    return out
